# revision 7
# baseline (speedup 1.0000x reference)
"""Balanced BCE loss on 8 Trainium2 NeuronCores.

loss = -sum_i [ beta_i * sum_j(t_ij * ln(p_ij))
                + (1-beta_i) * sum_j((1-t_ij) * ln(1-p_ij)) ]
beta_i = 1 - mean_j(t_ij)

Host casts inputs to bf16 (halves HBM traffic; quantization error on the
summed loss is ~1e-4 relative) and reshapes each core's 8 rows to a flat
[128, 16384] layout where row r owns partitions 16r..16r+15.

Per-core row statistics (8 batch rows per core):
  S=sum(t)  A=sum(t*lnp)  C=sum(t*ln1mp)  B=sum(ln1mp)
host combines: loss = -sum_rows[ beta*A + (1-beta)*(B-C) ], beta = 1-S/N

Engine assignment per column chunk [128, F] (graduated grid: big chunks
mid-stream for low instruction overhead, tiny last chunk for a short
tail):
  - ACT: lnp = Ln(p) bf16; ln1mp = Ln(1-p) bf16.  ACT is the bottleneck:
         2 passes = (2*16384 + ovh)/1.2GHz ~ 30us.
  - DVE: m1 = t*lnp, m2 = t*ln1mp (bf16 TT, 2x); S-reduces of t chunks.
  - PE: selector-matrix matmuls W[128,8]^T @ {m1, ln1mp, m2} accumulate
        A/B/C for all 8 rows at once into [8,128] PSUM accumulators
        (LDWEIGHTS hides under the previous matmul's drain).
  - Tail: 3 tiny [8,128] DVE reduces + one 128B output DMA.
"""

from contextlib import ExitStack

import numpy as np
import ml_dtypes

import concourse.bass as bass
import concourse.mybir as mybir
import concourse.tile as tile
from concourse import bacc
from concourse.bass_utils import run_bass_kernel_spmd

B, N = 64, 262144
NCORES = 8
ROWS = B // NCORES  # rows per core
P = 128  # SBUF partitions
NF = ROWS * N // P  # 16384 free-dim cols per partition
PPR = P // ROWS  # 16 partitions per row

AF = mybir.ActivationFunctionType
ALU = mybir.AluOpType
AX = mybir.AxisListType
f32 = mybir.dt.float32
bf16 = mybir.dt.bfloat16
np_bf16 = ml_dtypes.bfloat16

CH = 128  # PSUM accumulator width / matmul moving window
CHUNKS = [2048, 5120, 5120, 3584, 512]
assert sum(CHUNKS) == NF and all(c % CH == 0 for c in CHUNKS)

# test.py can flip this to capture an NTFF profile of the run
TRACE = False
LAST = None  # BassKernelResults of the most recent kernel() call


def _emit(tc, out_ap, inp_ap, tgt_ap, wf_ap):
    nc = tc.nc
    nch = len(CHUNKS)
    offs = [sum(CHUNKS[:i]) for i in range(nch)]
    nwin_total = NF // CH

    with ExitStack() as ctx:
        singles = ctx.enter_context(tc.tile_pool(name="s", bufs=1))
        psum_pool = ctx.enter_context(tc.tile_pool(name="ps", bufs=1, space="PSUM"))

        wf = singles.tile([P, ROWS], f32, tag="wf")
        wbf = singles.tile([P, ROWS], bf16, tag="wbf")
        accS = singles.tile([P, nch], f32, tag="accS")
        stats = singles.tile([ROWS, 4], f32, tag="stats")

        psA = psum_pool.tile([ROWS, CH], f32, tag="psA", name="psA")
        psB = psum_pool.tile([ROWS, CH], f32, tag="psB", name="psB")
        psC = psum_pool.tile([ROWS, CH], f32, tag="psC", name="psC")
        psS2 = psum_pool.tile([ROWS, nch], f32, tag="psS2", name="psS2")

        # stage all loads upfront on the SP queue; p chunks run one ahead
        # of t (ACT's critical path), selector W after p1 (PE needs it
        # only when t0's first products are ready)
        ptiles = [singles.tile([P, F], bf16, tag=f"p{c}", name=f"p{c}") for c, F in enumerate(CHUNKS)]
        ttiles = [singles.tile([P, F], bf16, tag=f"t{c}", name=f"t{c}") for c, F in enumerate(CHUNKS)]
        order = [("p", 0), ("p", 1), ("w", 0), ("t", 0), ("p", 2), ("t", 1),
                 ("p", 3), ("t", 2), ("p", 4), ("t", 3), ("t", 4)]
        for kind, c in order:
            if kind == "w":
                nc.sync.dma_start(wf[:], wf_ap)
                continue
            src = inp_ap if kind == "p" else tgt_ap
            dst = ptiles[c] if kind == "p" else ttiles[c]
            nc.sync.dma_start(dst[:], src[:, offs[c] : offs[c] + CHUNKS[c]])

        nc.vector.tensor_copy(wbf[:], wf[:])

        win = 0
        for c, F in enumerate(CHUNKS):
            p_t = ptiles[c][:]
            t_t = ttiles[c][:]

            lnp = singles.tile([P, F], bf16, tag=f"lnp{c}", name=f"lnp{c}")
            nc.scalar.activation(lnp[:], p_t, AF.Ln)
            l1mp = singles.tile([P, F], bf16, tag=f"l1mp{c}", name=f"l1mp{c}")
            nc.scalar.activation(l1mp[:], p_t, AF.Ln, scale=-1.0, bias=1.0)

            # S-reduce first: depends only on t_t, keeps the tail clear
            nc.vector.tensor_reduce(accS[:, c : c + 1], t_t, axis=AX.X, op=ALU.add)
            m1 = singles.tile([P, F], bf16, tag=f"m1{c}", name=f"m1{c}")
            nc.vector.tensor_mul(m1[:], t_t, lnp[:])
            m2 = singles.tile([P, F], bf16, tag=f"m2{c}", name=f"m2{c}")
            nc.vector.tensor_mul(m2[:], t_t, l1mp[:])

            for j in range(F // CH):
                sl = slice(j * CH, (j + 1) * CH)
                first = win == 0
                last = win == nwin_total - 1
                nc.tensor.matmul(psA[:, :], wbf[:], m1[:, sl], start=first, stop=last)
                nc.tensor.matmul(psB[:, :], wbf[:], l1mp[:, sl], start=first, stop=last)
                nc.tensor.matmul(psC[:, :], wbf[:], m2[:, sl], start=first, stop=last)
                win += 1

        # S fold: [128, nch] f32 -> [8, nch] on PE, emitted after the loop
        # but runs as soon as the last S-reduce lands (before ACT ends)
        nc.tensor.matmul(psS2[:, :], wf[:], accS[:, :])
        nc.vector.tensor_reduce(stats[:, 0:1], psS2[:, :], axis=AX.X, op=ALU.add)
        nc.vector.tensor_reduce(stats[:, 2:3], psA[:, :], axis=AX.X, op=ALU.add)
        nc.vector.tensor_reduce(stats[:, 1:2], psB[:, :], axis=AX.X, op=ALU.add)
        nc.vector.tensor_reduce(stats[:, 3:4], psC[:, :], axis=AX.X, op=ALU.add)
        nc.sync.dma_start(out_ap, stats[:])


_PROG_CACHE = {}


def _build_program():
    key = "v4"
    if key not in _PROG_CACHE:
        nc = bacc.Bacc("TRN2", target_bir_lowering=False, debug=False)
        inp = nc.dram_tensor("input", [P, NF], bf16, kind="ExternalInput").ap()
        tgt = nc.dram_tensor("target", [P, NF], bf16, kind="ExternalInput").ap()
        wf_d = nc.dram_tensor("wsel_f32", [P, ROWS], f32, kind="ExternalInput").ap()
        out = nc.dram_tensor("partials", [ROWS, 4], f32, kind="ExternalOutput").ap()
        with tile.TileContext(nc) as tc:
            _emit(tc, out, inp, tgt, wf_d)
        nc.finalize()
        _PROG_CACHE[key] = nc
    return _PROG_CACHE[key]


def kernel(input, target):
    global LAST
    input = np.asarray(input)
    target = np.asarray(target)
    assert input.shape == (B, N) and target.shape == (B, N)

    inp_bf = np.ascontiguousarray(input).astype(np_bf16)
    tgt_bf = np.ascontiguousarray(target).astype(np_bf16)

    nc = _build_program()
    wsel = np.zeros((P, ROWS), dtype=np.float32)
    for r in range(ROWS):
        wsel[r * PPR : (r + 1) * PPR, r] = 1.0
    in_maps = [
        {
            "input": inp_bf[c * ROWS : (c + 1) * ROWS].reshape(P, NF),
            "target": tgt_bf[c * ROWS : (c + 1) * ROWS].reshape(P, NF),
            "wsel_f32": wsel,
        }
        for c in range(NCORES)
    ]
    res = run_bass_kernel_spmd(nc, in_maps, core_ids=list(range(NCORES)), trace=TRACE)
    LAST = res

    total = np.float64(0.0)
    for c in range(NCORES):
        part = res.results[c]["partials"].astype(np.float64)  # [ROWS, 4]
        S, Bv, A, C = part[:, 0], part[:, 1], part[:, 2], part[:, 3]
        beta = 1.0 - S / N
        total += np.sum(beta * A + (1.0 - beta) * (Bv - C))
    return np.float32(-total)


# revision 9
# speedup vs baseline: 1.2664x; 1.2664x over previous
"""Balanced BCE loss on 8 Trainium2 NeuronCores.

loss = -sum_i [ beta_i * sum_j(t_ij * ln(p_ij))
                + (1-beta_i) * sum_j((1-t_ij) * ln(1-p_ij)) ]
beta_i = 1 - mean_j(t_ij)

Host casts inputs to bf16 (halves HBM traffic; quantization error on the
summed loss is ~1e-4 relative) and reshapes each core's 8 rows to a flat
[128, 16384] layout where row r owns partitions 16r..16r+15.

Reformulated with u = 1 - t:
  S' = sum_j(u)   A = sum_j(t * ln p)   C' = sum_j(u * ln(1-p))
  beta_i = S'_i / N ;  loss = -sum_rows[ beta*A + (1-beta)*C' ]
This needs exactly three row-reductions, all of tensors DVE already
produces, so no extra reduction pass exists anywhere.

Engine assignment per column chunk [128, F] (graduated grid: big chunks
mid-stream for low instruction overhead, small last chunk for a short
tail):
  - ACT: lnp = Ln(p) bf16; ln1mp = Ln(1-p) bf16.  ACT is the bottleneck:
         two passes over every element = (2*16384 + ovh)/1.2GHz ~ 30us.
  - DVE: u = 1-t (tensor_scalar, 4x); m1 = t*lnp, m2 = u*ln1mp (TT, 2x).
  - PE: selector-matrix matmuls W[128,8]^T @ {u, m1, m2} accumulate
        S'/A/C' for all 8 rows at once into [8,256] PSUM accumulators.
  - Tail: 3 tiny [8,256] DVE reduces + one 96B output DMA.
"""

from contextlib import ExitStack

import numpy as np
import ml_dtypes

import concourse.bass as bass
import concourse.mybir as mybir
import concourse.tile as tile
from concourse import bacc
from concourse.bass_utils import run_bass_kernel_spmd

B, N = 64, 262144
NCORES = 8
ROWS = B // NCORES  # rows per core
P = 128  # SBUF partitions
NF = ROWS * N // P  # 16384 free-dim cols per partition
PPR = P // ROWS  # 16 partitions per row

AF = mybir.ActivationFunctionType
ALU = mybir.AluOpType
AX = mybir.AxisListType
f32 = mybir.dt.float32
bf16 = mybir.dt.bfloat16
np_bf16 = ml_dtypes.bfloat16

CH = 256  # PSUM accumulator width / matmul moving window
CHUNKS = [2048, 5120, 5120, 3584, 512]
assert sum(CHUNKS) == NF and all(c % CH == 0 for c in CHUNKS)

# test.py can flip this to capture an NTFF profile of the run
TRACE = False
LAST = None  # BassKernelResults of the most recent kernel() call


def _emit(tc, out_ap, inp_ap, tgt_ap, wbf_ap):
    nc = tc.nc
    nch = len(CHUNKS)
    offs = [sum(CHUNKS[:i]) for i in range(nch)]
    nwin_total = NF // CH

    with ExitStack() as ctx:
        singles = ctx.enter_context(tc.tile_pool(name="s", bufs=1))
        psum_pool = ctx.enter_context(tc.tile_pool(name="ps", bufs=1, space="PSUM"))

        wbf = singles.tile([P, ROWS], bf16, tag="wbf")
        stats = singles.tile([ROWS, 3], f32, tag="stats")

        psS = psum_pool.tile([ROWS, CH], f32, tag="psS", name="psS")
        psA = psum_pool.tile([ROWS, CH], f32, tag="psA", name="psA")
        psC = psum_pool.tile([ROWS, CH], f32, tag="psC", name="psC")

        # stage all loads upfront on the SP queue; p chunks run one ahead
        # of t (ACT's critical path), selector W after p1 (PE needs it
        # only once t0's first products exist)
        ptiles = [singles.tile([P, F], bf16, tag=f"p{c}", name=f"p{c}") for c, F in enumerate(CHUNKS)]
        ttiles = [singles.tile([P, F], bf16, tag=f"t{c}", name=f"t{c}") for c, F in enumerate(CHUNKS)]
        order = [("p", 0), ("p", 1), ("w", 0), ("t", 0), ("p", 2), ("t", 1),
                 ("p", 3), ("t", 2), ("p", 4), ("t", 3), ("t", 4)]
        for kind, c in order:
            if kind == "w":
                nc.sync.dma_start(wbf[:], wbf_ap)
                continue
            src = inp_ap if kind == "p" else tgt_ap
            dst = ptiles[c] if kind == "p" else ttiles[c]
            nc.sync.dma_start(dst[:], src[:, offs[c] : offs[c] + CHUNKS[c]])

        win = 0
        for c, F in enumerate(CHUNKS):
            p_t = ptiles[c][:]
            t_t = ttiles[c][:]

            lnp = singles.tile([P, F], bf16, tag=f"lnp{c}", name=f"lnp{c}")
            nc.scalar.activation(lnp[:], p_t, AF.Ln)
            l1mp = singles.tile([P, F], bf16, tag=f"l1mp{c}", name=f"l1mp{c}")
            nc.scalar.activation(l1mp[:], p_t, AF.Ln, scale=-1.0, bias=1.0)

            # u first: depends only on t_t, keeps the tail clear
            u = singles.tile([P, F], bf16, tag=f"u{c}", name=f"u{c}")
            nc.vector.tensor_scalar(u[:], t_t, -1.0, 1.0, ALU.mult, ALU.add)
            # products overwrite the logs in place (their only consumer;
            # same-index elementwise on DVE is stream-safe)
            m1 = lnp
            nc.vector.tensor_mul(m1[:], t_t, lnp[:])
            m2 = l1mp
            nc.vector.tensor_mul(m2[:], u[:], l1mp[:])

            for j in range(F // CH):
                sl = slice(j * CH, (j + 1) * CH)
                first = win == 0
                last = win == nwin_total - 1
                nc.tensor.matmul(psS[:, :], wbf[:], u[:, sl], start=first, stop=last)
                nc.tensor.matmul(psA[:, :], wbf[:], m1[:, sl], start=first, stop=last)
                nc.tensor.matmul(psC[:, :], wbf[:], m2[:, sl], start=first, stop=last)
                win += 1

        nc.vector.tensor_reduce(stats[:, 0:1], psS[:, :], axis=AX.X, op=ALU.add)
        nc.vector.tensor_reduce(stats[:, 1:2], psA[:, :], axis=AX.X, op=ALU.add)
        nc.vector.tensor_reduce(stats[:, 2:3], psC[:, :], axis=AX.X, op=ALU.add)
        nc.sync.dma_start(out_ap, stats[:])


_PROG_CACHE = {}


def _build_program():
    key = "v5"
    if key not in _PROG_CACHE:
        nc = bacc.Bacc("TRN2", target_bir_lowering=False, debug=False)
        inp = nc.dram_tensor("input", [P, NF], bf16, kind="ExternalInput").ap()
        tgt = nc.dram_tensor("target", [P, NF], bf16, kind="ExternalInput").ap()
        wbf_d = nc.dram_tensor("wsel_bf", [P, ROWS], bf16, kind="ExternalInput").ap()
        out = nc.dram_tensor("partials", [ROWS, 3], f32, kind="ExternalOutput").ap()
        with tile.TileContext(nc) as tc:
            _emit(tc, out, inp, tgt, wbf_d)
        nc.finalize()
        _PROG_CACHE[key] = nc
    return _PROG_CACHE[key]


def kernel(input, target):
    global LAST
    input = np.asarray(input)
    target = np.asarray(target)
    assert input.shape == (B, N) and target.shape == (B, N)

    inp_bf = np.ascontiguousarray(input).astype(np_bf16)
    tgt_bf = np.ascontiguousarray(target).astype(np_bf16)

    nc = _build_program()
    wsel = np.zeros((P, ROWS), dtype=np.float32)
    for r in range(ROWS):
        wsel[r * PPR : (r + 1) * PPR, r] = 1.0
    wsel_bf = wsel.astype(np_bf16)
    in_maps = [
        {
            "input": inp_bf[c * ROWS : (c + 1) * ROWS].reshape(P, NF),
            "target": tgt_bf[c * ROWS : (c + 1) * ROWS].reshape(P, NF),
            "wsel_bf": wsel_bf,
        }
        for c in range(NCORES)
    ]
    res = run_bass_kernel_spmd(nc, in_maps, core_ids=list(range(NCORES)), trace=TRACE)
    LAST = res

    total = np.float64(0.0)
    for c in range(NCORES):
        part = res.results[c]["partials"].astype(np.float64)  # [ROWS, 3]
        Sp, A, Cp = part[:, 0], part[:, 1], part[:, 2]
        beta = Sp / N
        total += np.sum(beta * A + (1.0 - beta) * Cp)
    return np.float32(-total)


# revision 10
# speedup vs baseline: 1.2953x; 1.0229x over previous
"""Balanced BCE loss on 8 Trainium2 NeuronCores.

loss = -sum_i [ beta_i * sum_j(t_ij * ln(p_ij))
                + (1-beta_i) * sum_j((1-t_ij) * ln(1-p_ij)) ]
beta_i = 1 - mean_j(t_ij)

Host casts inputs to bf16 (halves HBM traffic; quantization error on the
summed loss is ~1e-4 relative) and reshapes each core's 8 rows to a flat
[128, 16384] layout where row r owns partitions 16r..16r+15.

Reformulated with u = 1 - t:
  S' = sum_j(u)   A = sum_j(t * ln p)   C' = sum_j(u * ln(1-p))
  beta_i = S'_i / N ;  loss = -sum_rows[ beta*A + (1-beta)*C' ]
This needs exactly three row-reductions, all of tensors DVE already
produces, so no extra reduction pass exists anywhere.

Engine assignment per column chunk [128, F] (graduated grid: big chunks
mid-stream for low instruction overhead, small last chunk for a short
tail):
  - ACT: lnp = Ln(p) bf16; ln1mp = Ln(1-p) bf16.  ACT is the bottleneck:
         two passes over every element = (2*16384 + ovh)/1.2GHz ~ 30us.
  - DVE: u = 1-t (tensor_scalar, 4x); m1 = t*lnp, m2 = u*ln1mp (TT, 2x).
  - PE: selector-matrix matmuls W[128,8]^T @ {u, m1, m2} accumulate
        S'/A/C' for all 8 rows at once into [8,256] PSUM accumulators.
  - Tail: 3 tiny [8,256] DVE reduces + one 96B output DMA.
"""

from contextlib import ExitStack

import numpy as np
import ml_dtypes

import concourse.bass as bass
import concourse.mybir as mybir
import concourse.tile as tile
from concourse import bacc
from concourse.bass_utils import run_bass_kernel_spmd

B, N = 64, 262144
NCORES = 8
ROWS = B // NCORES  # rows per core
P = 128  # SBUF partitions
NF = ROWS * N // P  # 16384 free-dim cols per partition
PPR = P // ROWS  # 16 partitions per row

AF = mybir.ActivationFunctionType
ALU = mybir.AluOpType
AX = mybir.AxisListType
f32 = mybir.dt.float32
bf16 = mybir.dt.bfloat16
np_bf16 = ml_dtypes.bfloat16

CH = 256  # PSUM accumulator width / matmul moving window
CHUNKS = [1024, 4096, 5632, 3840, 1536, 256]
assert sum(CHUNKS) == NF and all(c % CH == 0 for c in CHUNKS)

# test.py can flip this to capture an NTFF profile of the run
TRACE = False
LAST = None  # BassKernelResults of the most recent kernel() call


def _emit(tc, out_ap, inp_ap, tgt_ap, wbf_ap):
    nc = tc.nc
    nch = len(CHUNKS)
    offs = [sum(CHUNKS[:i]) for i in range(nch)]
    nwin_total = NF // CH

    with ExitStack() as ctx:
        singles = ctx.enter_context(tc.tile_pool(name="s", bufs=1))
        psum_pool = ctx.enter_context(tc.tile_pool(name="ps", bufs=1, space="PSUM"))

        wbf = singles.tile([P, ROWS], bf16, tag="wbf")
        stats = singles.tile([ROWS, 3], f32, tag="stats")

        psall = psum_pool.tile([ROWS, 3 * CH], f32, tag="psall", name="psall")
        psS = psall[:, 0 * CH : 1 * CH]
        psA = psall[:, 1 * CH : 2 * CH]
        psC = psall[:, 2 * CH : 3 * CH]

        # stage all loads upfront on the SP queue; p chunks run one ahead
        # of t (ACT's critical path), selector W after p1 (PE needs it
        # only once t0's first products exist)
        ptiles = [singles.tile([P, F], bf16, tag=f"p{c}", name=f"p{c}") for c, F in enumerate(CHUNKS)]
        ttiles = [singles.tile([P, F], bf16, tag=f"t{c}", name=f"t{c}") for c, F in enumerate(CHUNKS)]
        order = [("p", 0), ("p", 1), ("w", 0), ("t", 0), ("p", 2), ("t", 1),
                 ("p", 3), ("t", 2), ("p", 4), ("t", 3), ("p", 5), ("t", 4),
                 ("t", 5)]
        for kind, c in order:
            if kind == "w":
                nc.sync.dma_start(wbf[:], wbf_ap)
                continue
            src = inp_ap if kind == "p" else tgt_ap
            dst = ptiles[c] if kind == "p" else ttiles[c]
            nc.sync.dma_start(dst[:], src[:, offs[c] : offs[c] + CHUNKS[c]])

        win = 0
        for c, F in enumerate(CHUNKS):
            p_t = ptiles[c][:]
            t_t = ttiles[c][:]

            lnp = singles.tile([P, F], bf16, tag=f"lnp{c}", name=f"lnp{c}")
            nc.scalar.activation(lnp[:], p_t, AF.Ln)
            l1mp = singles.tile([P, F], bf16, tag=f"l1mp{c}", name=f"l1mp{c}")
            nc.scalar.activation(l1mp[:], p_t, AF.Ln, scale=-1.0, bias=1.0)

            # u first: depends only on t_t, keeps the tail clear
            u = singles.tile([P, F], bf16, tag=f"u{c}", name=f"u{c}")
            nc.vector.tensor_scalar(u[:], t_t, -1.0, 1.0, ALU.mult, ALU.add)
            # products overwrite the logs in place (their only consumer;
            # same-index elementwise on DVE is stream-safe)
            m1 = lnp
            nc.vector.tensor_mul(m1[:], t_t, lnp[:])
            m2 = l1mp
            nc.vector.tensor_mul(m2[:], u[:], l1mp[:])

            for ps, src_t in ((psS, u), (psA, m1), (psC, m2)):
                for j in range(F // CH):
                    sl = slice(j * CH, (j + 1) * CH)
                    first = win == 0
                    last = win + F // CH - (F // CH - 1) - 1 == nwin_total - 1 and j == F // CH - 1
                    nc.tensor.matmul(
                        ps, wbf[:], src_t[:, sl],
                        start=(win == 0 and j == 0),
                        stop=(win == nwin_total - CHUNKS[-1] // CH and j == F // CH - 1),
                        skip_group_check=True,
                    )
            win += F // CH

        nc.vector.tensor_reduce(
            stats[:, 0:3], psall[:].rearrange("p (s c) -> p s c", s=3),
            axis=AX.X, op=ALU.add,
        )
        nc.sync.dma_start(out_ap, stats[:])


_PROG_CACHE = {}


def _build_program():
    key = "v6"
    if key not in _PROG_CACHE:
        nc = bacc.Bacc("TRN2", target_bir_lowering=False, debug=False)
        inp = nc.dram_tensor("input", [P, NF], bf16, kind="ExternalInput").ap()
        tgt = nc.dram_tensor("target", [P, NF], bf16, kind="ExternalInput").ap()
        wbf_d = nc.dram_tensor("wsel_bf", [P, ROWS], bf16, kind="ExternalInput").ap()
        out = nc.dram_tensor("partials", [ROWS, 3], f32, kind="ExternalOutput").ap()
        with tile.TileContext(nc) as tc:
            _emit(tc, out, inp, tgt, wbf_d)
        nc.finalize()
        _PROG_CACHE[key] = nc
    return _PROG_CACHE[key]


def kernel(input, target):
    global LAST
    input = np.asarray(input)
    target = np.asarray(target)
    assert input.shape == (B, N) and target.shape == (B, N)

    inp_bf = np.ascontiguousarray(input).astype(np_bf16)
    tgt_bf = np.ascontiguousarray(target).astype(np_bf16)

    nc = _build_program()
    wsel = np.zeros((P, ROWS), dtype=np.float32)
    for r in range(ROWS):
        wsel[r * PPR : (r + 1) * PPR, r] = 1.0
    wsel_bf = wsel.astype(np_bf16)
    in_maps = [
        {
            "input": inp_bf[c * ROWS : (c + 1) * ROWS].reshape(P, NF),
            "target": tgt_bf[c * ROWS : (c + 1) * ROWS].reshape(P, NF),
            "wsel_bf": wsel_bf,
        }
        for c in range(NCORES)
    ]
    res = run_bass_kernel_spmd(nc, in_maps, core_ids=list(range(NCORES)), trace=TRACE)
    LAST = res

    total = np.float64(0.0)
    for c in range(NCORES):
        part = res.results[c]["partials"].astype(np.float64)  # [ROWS, 3]
        Sp, A, Cp = part[:, 0], part[:, 1], part[:, 2]
        beta = Sp / N
        total += np.sum(beta * A + (1.0 - beta) * Cp)
    return np.float32(-total)
